# revision 62
# baseline (speedup 1.0000x reference)
"""Self-contained Trainium2 Bass kernel for a single attention head.

Reference computation (per batch b):
    Q = x @ Wq + bq ; K = x @ Wk + bk ; V = x @ Wv + bv      (x: [S, M])
    out = softmax(Q K^T / sqrt(D)) @ V                        ([S, D])

Shapes: B=4, S=4096, M=1024, D=128, f32.

Sharding (key-split + host merge): 8 cores; core c handles batch b=c//2 and
KEY-half h=c%2. Each core projects K/V for its own 2048 key rows only, Q for
all 4096 queries, and computes the UNNORMALIZED partial attention
  N_h^T[dv, q] = sum_{s in half h} exp(q.k_s/sqrt(D)) v_s,   d_h[q] = sum_s exp(.)
over its key half. The host merges: O = (N_0 + N_1) / (d_0 + d_1) — softmax
over the key axis is an exact sum-decomposition, so the merge is exact. This
halves the redundant K/V projection work and removes every on-device
normalization/transpose step (the host divides and transposes). The host
permutes each core's rows so its key half comes first (key order inside a
softmax is irrelevant; the query order is un-permuted on the host).

Device pipeline (fully fused stream over 8 x-chunks of 512 rows):
  - chunks 0-3 project K^T/V (own keys) + Q^T; chunks 4-7 project Q^T only.
    x is bf16 (host-converted; halves DMA). V is built in natural [s, dv]
    layout directly via x-tile-stationary matmuls (no transposes).
  - 8 q-chunks chase the stream: scores S^T[s,q] (bf16 matmuls into PSUM),
    one wide exp per s-tile pair [128,1024] -> A^T bf16, attn@V accumulates
    N^T in PSUM. Two q-chunks hold the two O-psum banks at a time; the
    others defer attn@V until a slot frees (their A^T stays in SBUF).
  - denominator: per q-chunk DVE bf16 tree -> f32 -> GPSIMD
    partition_all_reduce -> DMA; the last q-chunk accumulates its den with
    PE ones-matmuls instead so the post-last-exp tail is tiny.
  - N^T q-slabs DMA out straight from the PSUM drain; no finalize pass.
  - PSUM (8 banks): scores 2x2 + N^T accumulators 2 + projections 2.
"""

from contextlib import ExitStack

import numpy as np

import concourse.bass as bass
import concourse.tile as tile
from concourse import bacc, mybir
from concourse.bass_utils import run_bass_kernel_spmd
from concourse.masks import make_identity

F32 = mybir.dt.float32
BF16 = mybir.dt.bfloat16

B, S, M, D = 4, 4096, 1024, 128
N_CORES = 8
P = 128
SCALE = 1.0 / np.sqrt(np.float32(D))


def build_attention(nc, S_all=S, M_dim=M, SC=512, QC=512):
    KH = S_all // 2               # keys per core (2048)
    MT = M_dim // P               # m-tiles (8)
    ST = KH // P                  # key s-tiles (16)
    NSC = S_all // SC             # x-chunks (8)
    NKC = KH // SC                # key chunks (4)
    SCT = SC // P                 # s-tiles per chunk (4)
    NQC = S_all // QC             # q-chunks (8)
    NPAIR = ST // 2               # score pairs per q-chunk (8)
    LQ = NQC - 1                  # last q-chunk (PE-ones denominator)

    xT = nc.dram_tensor("xT", [M_dim, S_all], BF16, kind="ExternalInput").ap()
    wq = nc.dram_tensor("wq", [P, MT * D], BF16, kind="ExternalInput").ap()
    wk = nc.dram_tensor("wk", [P, MT * D], BF16, kind="ExternalInput").ap()
    wv = nc.dram_tensor("wv", [P, MT * D], BF16, kind="ExternalInput").ap()
    bkq = nc.dram_tensor("bkq", [P, 2], F32, kind="ExternalInput").ap()
    bv = nc.dram_tensor("bv", [1, D], F32, kind="ExternalInput").ap()
    out = nc.dram_tensor("out", [D, S_all], BF16, kind="ExternalOutput").ap()
    den = nc.dram_tensor("den", [1, S_all], F32, kind="ExternalOutput").ap()

    xT_r = xT.rearrange("(t p) s -> p t s", p=P)

    Ident = mybir.ActivationFunctionType.Identity
    Exp = mybir.ActivationFunctionType.Exp

    with tile.TileContext(nc) as tc:
        ctx = ExitStack()
        persist = ctx.enter_context(tc.tile_pool(name="persist", bufs=1))

        ident = persist.tile([P, P], F32)
        make_identity(nc, ident[:])
        identb = persist.tile([P, P], BF16)
        nc.vector.tensor_copy(identb[:], ident[:])
        ones_f = persist.tile([P, 1], F32)
        nc.vector.memset(ones_f[:], 1.0)
        onesb = persist.tile([P, 1], BF16)
        nc.vector.tensor_copy(onesb[:], ones_f[:])

        # startup DMA order: wk (gpsimd queue, instant issue), then biases +
        # x chunk 0 halves + wq/wv on the SP queue in priority order
        xstage = ctx.enter_context(tc.tile_pool(name="xstage", bufs=2))
        wk_sb = persist.tile([P, MT, D], BF16)
        nc.gpsimd.dma_start(wk_sb[:], wk.rearrange("p (t d) -> p t d", d=D))
        bkq_sb = persist.tile([P, 2], F32)
        nc.scalar.dma_start(bkq_sb[:], bkq)
        bk_sb = bkq_sb[:, 0:1]
        bq_sb = bkq_sb[:, 1:2]
        bv_row = persist.tile([1, D], F32)
        nc.scalar.dma_start(bv_row[:], bv)
        bv_bcast = persist.tile([P, D], F32)
        nc.gpsimd.partition_broadcast(bv_bcast[:], bv_row[:])
        x_r0 = xstage.tile([P, MT, SC], BF16, name="x_r")
        nc.sync.dma_start(x_r0[:, 0:MT // 2, :],
                          xT_r[:, 0:MT // 2, bass.ds(0, SC)])
        nc.sync.dma_start(x_r0[:, MT // 2:, :],
                          xT_r[:, MT // 2:, bass.ds(0, SC)])
        wq_sb = persist.tile([P, MT, D], BF16)
        nc.sync.dma_start(wq_sb[:], wq.rearrange("p (t d) -> p t d", d=D))
        wv_sb = persist.tile([P, MT, D], BF16)
        nc.sync.dma_start(wv_sb[:], wv.rearrange("p (t d) -> p t d", d=D))

        kT_sb = persist.tile([P, KH], BF16)        # K^T  [dk, s]
        qT_sb = persist.tile([P, S_all], BF16)     # Q^T  [dk, q]
        v_sb = persist.tile([P, ST, D], BF16)      # V    [s%128, s-tile, dv]

        apool = ctx.enter_context(tc.tile_pool(name="apool", bufs=5))
        dpool = ctx.enter_context(tc.tile_pool(name="dpool", bufs=2))
        otpool = ctx.enter_context(tc.tile_pool(name="otpool", bufs=2))
        spsum = ctx.enter_context(tc.tile_pool(name="spsum", bufs=2, space="PSUM"))
        opsum = ctx.enter_context(tc.tile_pool(name="opsum", bufs=2, space="PSUM"))

        a_t = {}      # qc -> A^T tile [P, ST, QC] bf16
        o_ps = {}     # qc -> N^T psum [P, QC]
        t1 = {}       # qc -> den partial [P, 4, QC] bf16
        dall = {}     # qc -> all-reduced partial denominator [P, QC] f32
        tail = {}     # LQ's PE-ones den psum [1, QC]

        def qsl(qc):
            return bass.ds(qc * QC, QC)

        def emit_scores(qc, pr, split_exp=False):
            """Scores for s-tiles (2pr, 2pr+1) x q-chunk qc + one wide exp."""
            if pr == 0:
                a_t[qc] = apool.tile([P, ST, QC], BF16, name="a_sb")
            ps_s = spsum.tile([P, 2, QC], F32, name="ps_s")
            for j in range(2):
                st = 2 * pr + j
                nc.tensor.matmul(ps_s[:, j, :], kT_sb[:, bass.ts(st, P)],
                                 qT_sb[:, qsl(qc)], start=True, stop=True)
            if split_exp:
                for j in range(2):
                    st = 2 * pr + j
                    nc.scalar.activation(a_t[qc][:, st:st + 1, :],
                                         ps_s[:, j:j + 1, :], Exp,
                                         scale=float(SCALE))
            else:
                nc.scalar.activation(a_t[qc][:, 2 * pr:2 * pr + 2, :], ps_s[:],
                                     Exp, scale=float(SCALE))
            # denominator tree triggers. LQ folds tiles 0..11 early (its last
            # 4 tiles ride the PE ones-matmuls so the end tail stays short).
            a = a_t[qc]
            if pr == 3:
                t1[qc] = dpool.tile([P, 4, QC], BF16, name="t1")
                nc.vector.tensor_add(t1[qc][:], a[:, 0:4, :], a[:, 4:8, :])
            elif qc != LQ and pr == 7:
                tb = dpool.tile([P, 4, QC], BF16, name="tb")
                nc.vector.tensor_add(tb[:], a[:, 8:12, :], a[:, 12:16, :])
                nc.vector.tensor_add(t1[qc][:], t1[qc][:], tb[:])
                _den_fold(qc)
                nc.sync.dma_start(den[:, qsl(qc)], dall[qc][:1, :])
            elif qc == LQ and pr == 5:
                nc.vector.tensor_add(t1[qc][:], t1[qc][:], a[:, 8:12, :])
                _den_fold(qc)

        def _den_fold(qc):
            nc.vector.tensor_add(t1[qc][:, 0:2, :], t1[qc][:, 0:2, :],
                                 t1[qc][:, 2:4, :])
            den128 = dpool.tile([P, QC], F32, name="den128")
            nc.vector.tensor_add(den128[:], t1[qc][:, 0, :], t1[qc][:, 1, :])
            dl = dpool.tile([P, QC], F32, name="dall")
            nc.gpsimd.partition_all_reduce(dl[:], den128[:], P,
                                           bass.bass_isa.ReduceOp.add)
            dall[qc] = dl

        def emit_av(qc, pr):
            """attn@V accumulation for pair pr; the last q-chunk's final four
            s-tiles also feed the PE ones-matmul denominator tail."""
            if qc not in o_ps:
                o_ps[qc] = opsum.tile([P, QC], F32, name="o_ps")
            for j in range(2):
                st = 2 * pr + j
                nc.tensor.matmul(o_ps[qc][:], v_sb[:, st, :],
                                 a_t[qc][:, st, :],
                                 start=(st == 0), stop=(st == ST - 1))
            if qc == LQ and pr >= NPAIR - 2:
                if pr == NPAIR - 2:
                    tail[qc] = opsum.tile([1, QC], F32, name="o_ps")
                for j in range(2):
                    st = 2 * pr + j
                    nc.tensor.matmul(tail[qc][:], onesb[:], a_t[qc][:, st, :],
                                     start=(st == ST - 4), stop=(st == ST - 1))

        def finish_qc(qc):
            """Drain N^T to SBUF and DMA it out; LQ also drains its PE-ones
            denominator (on ACT, parallel to the DVE drain). The last drain
            and output DMA are split in halves so the first transfer's launch
            latency overlaps the second half's drain."""
            oT = otpool.tile([P, QC], BF16, name="oT")
            if qc == LQ:
                tail_sb = dpool.tile([1, QC], F32, name="tail_sb", bufs=1)
                nc.scalar.copy(tail_sb[:], tail[qc][:])
                dcomb = dpool.tile([1, QC], F32, name="dcomb", bufs=1)
                nc.vector.tensor_add(dcomb[:], dall[qc][:1, :], tail_sb[:])
                nc.gpsimd.dma_start(den[:, qsl(qc)], dcomb[:])
                h = QC // 2
                nc.vector.tensor_copy(oT[:, 0:h], o_ps[qc][:, 0:h])
                nc.sync.dma_start(out[:, bass.ds(qc * QC, h)], oT[:, 0:h])
                nc.vector.tensor_copy(oT[:, h:], o_ps[qc][:, h:])
                # second half on the ACT queue so its launch overlaps the first
                nc.scalar.dma_start(out[:, bass.ds(qc * QC + h, h)], oT[:, h:])
            else:
                nc.vector.tensor_copy(oT[:], o_ps[qc][:])
                nc.sync.dma_start(out[:, qsl(qc)], oT[:])

        # scheduler state: two q-chunks own the O-psum banks at a time; the
        # rest chase scores only and catch up attn@V when promoted.
        sc_done = [0] * NQC
        av_done = [0] * NQC
        full = [0, 1]
        next_full = [2]

        def pump_avs(qc):
            if qc not in full:
                return
            while av_done[qc] < sc_done[qc] - (1 if sc_done[qc] < NPAIR else 0):
                emit_av(qc, av_done[qc])
                av_done[qc] += 1
            if av_done[qc] == NPAIR:
                finish_qc(qc)
                full.remove(qc)
                if next_full[0] < NQC:
                    nq = next_full[0]
                    next_full[0] += 1
                    full.append(nq)
                    pump_avs(nq)

        # PE warm-up: back-to-back transposes ramp the PE p-state while the
        # first x chunk lands.
        with tc.tile_pool(name="warm", bufs=1, space="PSUM") as wp:
            warm_ps = wp.tile([P, P], BF16, name="warm_ps")
            for _ in range(28):
                nc.tensor.transpose(warm_ps[:], identb[:], identb[:])

        with tc.tile_pool(name="pp", bufs=2, space="PSUM") as pp:
            for sc in range(NSC):
                ssl = bass.ds(sc * SC, SC)
                if sc == 0:
                    x_r = x_r0
                else:
                    x_r = xstage.tile([P, MT, SC], BF16, name="x_r")
                    nc.sync.dma_start(x_r[:], xT_r[:, :, ssl])

                if sc < NKC:
                    # K^T chunk (own key half = permuted row prefix)
                    ps = pp.tile([P, SC], F32, name="pp")
                    for mt in range(MT):
                        nc.tensor.matmul(ps[:], wk_sb[:, mt, :], x_r[:, mt, :],
                                         start=(mt == 0), stop=(mt == MT - 1))
                    nc.vector.tensor_scalar_add(kT_sb[:, ssl], ps[:], bk_sb)

                # Q^T chunk (all 8 chunks); late-chunk drains go on the DVE so
                # they don't stretch the exp cadence in the ACT-bound phase
                ps2 = pp.tile([P, SC], F32, name="pp")
                for mt in range(MT):
                    nc.tensor.matmul(ps2[:], wq_sb[:, mt, :], x_r[:, mt, :],
                                     start=(mt == 0), stop=(mt == MT - 1))
                nc.vector.tensor_scalar_add(qT_sb[:, ssl], ps2[:], bq_sb)

                # first chaser's new scores go ahead of the V projection so
                # the ACT exp queue is fed across the chunk boundary
                avail = min(NKC, sc + 1) * SCT // 2
                first = next((q for q in range(NQC)
                              if q <= sc and sc_done[q] < avail), None)
                if first is not None:
                    while sc_done[first] < avail:
                        pr = sc_done[first]
                        emit_scores(first, pr,
                                    split_exp=(first == LQ and
                                               pr == NPAIR - 1))
                        sc_done[first] += 1

                if sc < NKC:
                    # V chunk in natural [s, dv] layout: x-tile stationary
                    for t in range(SCT):
                        st = sc * SCT + t
                        psv = pp.tile([P, D], F32, name="pp")
                        for mt in range(MT):
                            nc.tensor.matmul(psv[:], x_r[:, mt, bass.ts(t, P)],
                                             wv_sb[:, mt, :],
                                             start=(mt == 0),
                                             stop=(mt == MT - 1))
                        nc.vector.tensor_add(v_sb[:, st, :], psv[:],
                                             bv_bcast[:])

                # chase: remaining q-chunks catch up; slot owners run attn@V
                for qc in range(NQC):
                    if qc <= sc:
                        while sc_done[qc] < avail:
                            pr = sc_done[qc]
                            emit_scores(qc, pr,
                                        split_exp=(qc == LQ and
                                                   pr == NPAIR - 1))
                            sc_done[qc] += 1
                            if qc in full:
                                pump_avs(qc)
                        if qc in full:
                            pump_avs(qc)

        # post-stream: everything has its scores emitted; drain the rest
        for qc in list(full):
            pump_avs(qc)
        while full:
            qc = full[0]
            pump_avs(qc)
        ctx.close()

    return nc


def build(n_cores=N_CORES, **kw):
    nc = bacc.Bacc("TRN2", target_bir_lowering=False, debug=False,
                   num_devices=n_cores)
    build_attention(nc, **kw)
    nc.compile()
    return nc


def shard_inputs(input, Wq, bq, Wk, bk, Wv, bv):
    """Per-core in_maps. Core c: batch c//2, key-half c%2; the host permutes
    the batch so the core's key rows come first, transposes to xT [M, S],
    and converts to bf16. Weights are packed [P, MT*D]; biases packed small."""
    import ml_dtypes
    half = S // 2
    MT = M // 128

    def pack_w(W):
        return np.ascontiguousarray(
            np.asarray(W, dtype=np.float32).reshape(MT, 128, D)
            .transpose(1, 0, 2).reshape(128, MT * D)).astype(ml_dtypes.bfloat16)

    wq_b, wk_b, wv_b = pack_w(Wq), pack_w(Wk), pack_w(Wv)
    bkq_f = np.ascontiguousarray(np.stack(
        [np.asarray(bk, dtype=np.float32).ravel(),
         np.asarray(bq, dtype=np.float32).ravel()], axis=1))
    bv_f = np.asarray(bv, dtype=np.float32).reshape(1, D)
    in_maps = []
    for c in range(N_CORES):
        b, h = divmod(c, 2)
        xb = np.asarray(input[b])
        x_perm = np.concatenate(
            [xb[h * half:(h + 1) * half], xb[(1 - h) * half:(2 - h) * half]],
            axis=0)
        xT = np.ascontiguousarray(x_perm.T).astype(ml_dtypes.bfloat16)
        in_maps.append({
            "xT": xT,
            "wq": wq_b, "wk": wk_b, "wv": wv_b,
            "bkq": bkq_f, "bv": bv_f,
        })
    return in_maps


_NC_CACHE = {}


def kernel(input, Wq, bq, Wk, bk, Wv, bv):
    in_maps = shard_inputs(input, Wq, bq, Wk, bk, Wv, bv)
    if "nc" not in _NC_CACHE:
        _NC_CACHE["nc"] = build()
    nc = _NC_CACHE["nc"]
    res = run_bass_kernel_spmd(nc, in_maps, core_ids=list(range(N_CORES)))
    half = S // 2
    result = np.empty((B, S, D), dtype=np.float32)
    for b in range(B):
        # core 2b: keys half0, rows in natural order
        # core 2b+1: keys half1, rows permuted [half1; half0]
        n0 = np.asarray(res.results[2 * b]["out"]).astype(np.float32).T
        d0 = np.asarray(res.results[2 * b]["den"]).astype(np.float32).ravel()
        n1p = np.asarray(res.results[2 * b + 1]["out"]).astype(np.float32).T
        d1p = np.asarray(res.results[2 * b + 1]["den"]).astype(np.float32).ravel()
        n1 = np.concatenate([n1p[half:], n1p[:half]], axis=0)
        d1 = np.concatenate([d1p[half:], d1p[:half]])
        result[b] = (n0 + n1) / (d0 + d1)[:, None]
    return result


if __name__ == "__main__":
    rng = np.random.default_rng(0)
    inputs = {
        "input": rng.standard_normal((B, S, M), dtype=np.float32),
        "Wq": (rng.standard_normal((M, D), dtype=np.float32) / np.sqrt(M)).astype(np.float32),
        "bq": (rng.standard_normal(D, dtype=np.float32) * 0.02),
        "Wk": (rng.standard_normal((M, D), dtype=np.float32) / np.sqrt(M)).astype(np.float32),
        "bk": (rng.standard_normal(D, dtype=np.float32) * 0.02),
        "Wv": (rng.standard_normal((M, D), dtype=np.float32) / np.sqrt(M)).astype(np.float32),
        "bv": (rng.standard_normal(D, dtype=np.float32) * 0.02),
    }
    out = kernel(**inputs)
    print("kernel output:", out.shape, out.dtype)
